# revision 24
# baseline (speedup 1.0000x reference)
"""Bipartite GCN node-selection policy on 8 trn2 NeuronCores (Bass/Tile).

Strategy:
- Algebraic restructure: the edge-MLP first layer splits into per-node
  projections (P_src[src] + P_dst[dst] + ea*u + b), and the second GEMM +
  bias commute past segment_sum (agg = R @ g2W + cnt x g2b). Per-edge work
  is then: 2 row gathers + fused elementwise + relu + one-hot scatter-matmul.
- Sharding: edges partitioned by destination-node shard (8 ways), ordered by
  (src-table-chunk, dst-window). Each core owns 1/8 of destination nodes and
  accumulates R^T locally in PSUM/SBUF. One AllGather for the conv2 source
  table; tiny AllReduces for BN stats and the final mean-pool.
- Gathers via dma_gather (int16 idx, bf16 tables at 256B row pitch).
"""
import numpy as np
import ml_dtypes

import concourse.bass as bass
import concourse.bacc as bacc
import concourse.tile as tile
import concourse.mybir as mybir
from concourse.bass_utils import run_bass_kernel_spmd

BF16 = ml_dtypes.bfloat16
F32 = mybir.dt.float32
BF = mybir.dt.bfloat16
I16 = mybir.dt.int16
AF = mybir.ActivationFunctionType
OP = mybir.AluOpType
AX = mybir.AxisListType

H = 64
NV, NCN, E = 100000, 50000, 1600000
NCORES = 8
SHC, SHV = NCN // NCORES, NV // NCORES     # 6250, 12500
WIN = 128
CHUNK = 25000
GT = 16                                     # tiles per gather group
BN_EPS = 1e-5
P = 128


# ----------------------------------------------------------------------------
# Host-side edge preparation
# ----------------------------------------------------------------------------

def _prep_conv(dst, src, ea, sh, n_chunks):
    """Shard edges by dst//sh, order by (src-chunk, dst-window), pad runs to a
    cross-core-common tile structure, emit per-group streams."""
    nwin = (sh + WIN - 1) // WIN
    shard = dst // sh
    per_core_raw = []
    counts = np.zeros((NCORES, n_chunks * nwin), dtype=np.int64)
    for k in range(NCORES):
        sel = np.nonzero(shard == k)[0]
        d_loc = (dst[sel] - k * sh).astype(np.int32)
        s_glob = src[sel].astype(np.int32)
        key = (s_glob // CHUNK) * nwin + (d_loc // WIN)
        order = np.argsort(key, kind="stable")
        d_loc, s_glob = d_loc[order], s_glob[order]
        counts[k] = np.bincount(key[order], minlength=n_chunks * nwin)
        per_core_raw.append((d_loc, s_glob, ea[sel[order]].astype(np.float32)))

    tiles_cw = (counts.max(axis=0) + WIN - 1) // WIN   # [n_chunks*nwin]
    runs = []          # (chunk, win, t0, ntiles)
    t_acc = 0
    for c in range(n_chunks):
        for w in range(nwin):
            nt = int(tiles_cw[c * nwin + w])
            if nt == 0:
                continue
            runs.append((c, w, t_acc, nt))
            t_acc += nt
    T = t_acc

    groups = []        # (chunk, t0, ntiles)
    for c in range(n_chunks):
        c_runs = [r for r in runs if r[0] == c]
        if not c_runs:
            continue
        c0, c1 = c_runs[0][2], c_runs[-1][2] + c_runs[-1][3]
        t = c0
        while t < c1:
            nt = min(GT, c1 - t)
            groups.append((c, t, nt))
            t += nt
    NG = len(groups)

    seen = set()
    first_visit = []
    for (c, w, t0, nt) in runs:
        first_visit.append(w not in seen)
        seen.add(w)

    per_core = []
    for k in range(NCORES):
        d_loc, s_glob, ea_k = per_core_raw[k]
        src16 = np.zeros(T * WIN, dtype=np.int16)
        dst16 = np.zeros(T * WIN, dtype=np.int16)
        rel = np.full(T * WIN, -1.0, dtype=np.float32)
        eap = np.zeros(T * WIN, dtype=np.float32)
        cw_start = np.zeros(n_chunks * nwin + 1, dtype=np.int64)
        np.cumsum(counts[k], out=cw_start[1:])
        for (c, w, t0, nt) in runs:
            i0, i1 = cw_start[c * nwin + w], cw_start[c * nwin + w + 1]
            n = i1 - i0
            base = t0 * WIN
            src16[base:base + n] = (s_glob[i0:i1] % CHUNK).astype(np.int16)
            dst16[base:base + n] = d_loc[i0:i1].astype(np.int16)
            rel[base:base + n] = (d_loc[i0:i1] % WIN).astype(np.float32)
            eap[base:base + n] = ea_k[i0:i1]

        srcw = np.zeros((NG, P, GT * 8), dtype=np.int16)
        dstw = np.zeros((NG, P, GT * 8), dtype=np.int16)
        eag = np.zeros((NG, P, GT), dtype=np.float32)
        relg = np.full((NG, P, GT), -1.0, dtype=np.float32)
        for gi, (c, t0, nt) in enumerate(groups):
            blk_s = src16[t0 * WIN:(t0 + nt) * WIN]
            blk_d = dst16[t0 * WIN:(t0 + nt) * WIN]
            w_s = blk_s.reshape(-1, 16).T                 # [16, nt*8]
            w_d = blk_d.reshape(-1, 16).T
            srcw[gi, :, :nt * 8] = np.tile(w_s, (8, 1))
            dstw[gi, :, :nt * 8] = np.tile(w_d, (8, 1))
            eag[gi, :, :nt] = eap[t0 * WIN:(t0 + nt) * WIN].reshape(nt, WIN).T
            relg[gi, :, :nt] = rel[t0 * WIN:(t0 + nt) * WIN].reshape(nt, WIN).T

        cnt = np.bincount(d_loc, minlength=sh).astype(np.float32)[None, :]
        per_core.append(dict(srcw=srcw, dstw=dstw, ea=eag,
                             rel=relg.astype(BF16), cnt=cnt))
    common = dict(runs=runs, groups=groups, T=T, nwin=nwin,
                  first_visit=first_visit, n_chunks=n_chunks, sh=sh, NG=NG)
    return common, per_core


def _np(x):
    return np.asarray(x, dtype=np.float32)


def _prep_weights(params):
    w = {}
    w["vW"] = _np(params["var_init"]["W"])
    w["vb"] = _np(params["var_init"]["b"]).reshape(H, 1)
    w["cW"] = _np(params["con_init"]["W"])
    w["cb"] = _np(params["con_init"]["b"]).reshape(H, 1)
    for j, key in ((1, "v_to_c"), (2, "c_to_v")):
        p = params[key]
        g1W = _np(p["g1W"])
        w[f"g1Wa{j}"] = g1W[:H].copy()
        w[f"g1Wb{j}"] = g1W[H:2 * H].copy()
        w[f"g1b{j}"] = _np(p["g1b"]).reshape(1, H)
        w[f"ug{j}"] = np.tile(g1W[2 * H], (P, 1)).astype(BF16)
        w[f"g2W{j}"] = _np(p["g2W"])
        w[f"g2b{j}"] = _np(p["g2b"]).reshape(1, H)
        w[f"bng{j}"] = _np(p["bn_g"]).reshape(H, 1)
        w[f"bnb{j}"] = _np(p["bn_b"]).reshape(H, 1)
        f1W = _np(p["f1W"])
        w[f"f1Wt{j}"] = f1W[:H].copy()
        w[f"f1Wb{j}"] = f1W[H:].copy()
        w[f"f1b{j}"] = _np(p["f1b"]).reshape(H, 1)
        w[f"f2W{j}"] = _np(p["f2W"])
        w[f"f2b{j}"] = _np(p["f2b"]).reshape(1, H)
    hp = params["head"]
    w["hW1"] = _np(hp["W1"])
    w["hb1"] = _np(hp["b1"]).reshape(H, 1)
    w["hW2"] = _np(hp["W2"]).reshape(H, 1)
    w["hb2"] = _np(hp["b2"]).reshape(1, 1)
    w["iota"] = np.tile(np.arange(P).astype(BF16), (P, 1))
    return w


# ----------------------------------------------------------------------------
# Device program
# ----------------------------------------------------------------------------

def _edge_phase(nc, tc, work, psb, com, ins, src_tab_views, dst_tab, ug, iota,
                rt_sb):
    """Edge phase: per-group streamed gathers + msg + one-hot scatter-matmul
    into rt_sb [64, nwin*128] (f32)."""
    run_of_tile = {}
    for ri, (c, w, t0, nt) in enumerate(com["runs"]):
        for t in range(t0, t0 + nt):
            run_of_tile[t] = ri

    psum_of_run = {}
    for gi, (c, g0, gnt) in enumerate(com["groups"]):
        ni = gnt * WIN
        srcw_g = work.tile([P, GT * 8], I16, tag="srcw_g")
        dstw_g = work.tile([P, GT * 8], I16, tag="dstw_g")
        ea_g = work.tile([P, GT], F32, tag="ea_g")
        rel_g = work.tile([P, GT], BF, tag="rel_g")
        nc.sync.dma_start(out=srcw_g[:], in_=ins["srcw"].ap()[gi, :, :])
        nc.sync.dma_start(out=dstw_g[:], in_=ins["dstw"].ap()[gi, :, :])
        nc.sync.dma_start(out=ea_g[:], in_=ins["ea"].ap()[gi, :, :])
        nc.sync.dma_start(out=rel_g[:], in_=ins["rel"].ap()[gi, :, :])

        gs = work.tile([P, GT, 128], BF, tag="gs")
        gd = work.tile([P, GT, 128], BF, tag="gd")
        nc.gpsimd.dma_gather(
            out_ap=gs[:, :gnt, :], in_ap=src_tab_views[c],
            idxs_ap=srcw_g[:, :gnt * 8],
            num_idxs=ni, num_idxs_reg=ni, elem_size=128, single_packet=False)
        nc.gpsimd.dma_gather(
            out_ap=gd[:, :gnt, :], in_ap=dst_tab,
            idxs_ap=dstw_g[:, :gnt * 8],
            num_idxs=ni, num_idxs_reg=ni, elem_size=128, single_packet=False)

        s_t = work.tile([P, GT, H], BF, tag="s_t")
        nc.vector.tensor_tensor(out=s_t[:, :gnt, :], in0=gs[:, :gnt, :H],
                                in1=gd[:, :gnt, :H], op=OP.add)
        pre = work.tile([P, GT, H], BF, tag="pre")
        for ti in range(gnt):
            nc.vector.scalar_tensor_tensor(
                out=pre[:, ti, :], in0=ug[:], scalar=ea_g[:, ti:ti + 1],
                in1=s_t[:, ti, :], op0=OP.mult, op1=OP.add)
        msg = work.tile([P, GT, H], BF, tag="msg")
        nc.scalar.activation(
            out=msg[:, :gnt, :].rearrange("p t d -> p (t d)"),
            in_=pre[:, :gnt, :].rearrange("p t d -> p (t d)"), func=AF.Relu)
        oh = work.tile([P, GT, 128], BF, tag="oh")
        nc.vector.tensor_tensor(
            out=oh[:, :gnt, :],
            in0=iota[:].unsqueeze(1).to_broadcast([P, gnt, 128]),
            in1=rel_g[:, :gnt].unsqueeze(2).to_broadcast([P, gnt, 128]),
            op=OP.is_equal)

        for ti in range(gnt):
            t = g0 + ti
            ri = run_of_tile[t]
            c_r, w_r, t0_r, nt_r = com["runs"][ri]
            if t == t0_r:
                psum_of_run[ri] = psb.tile([H, 128], F32, space="PSUM",
                                           tag="rt", name="rtps")
            rp = psum_of_run[ri]
            nc.tensor.matmul(out=rp[:], lhsT=msg[:, ti, :], rhs=oh[:, ti, :],
                             start=(t == t0_r), stop=(t == t0_r + nt_r - 1))
            if t == t0_r + nt_r - 1:
                span = rt_sb[:, w_r * 128:(w_r + 1) * 128]
                if com["first_visit"][ri]:
                    nc.vector.tensor_copy(out=span, in_=rp[:])
                else:
                    nc.vector.tensor_tensor(out=span, in0=span, in1=rp[:],
                                            op=OP.add)
                del psum_of_run[ri]


def _bn_coeffs(nc, sb, stats, bng, bnb, n_total, tag):
    m = sb.tile([H, 2], F32, tag=f"{tag}_m")
    nc.vector.tensor_scalar_mul(m[:], stats[:], 1.0 / n_total)
    msq = sb.tile([H, 1], F32, tag=f"{tag}_msq")
    nc.vector.tensor_tensor(out=msq[:], in0=m[:, 0:1], in1=m[:, 0:1], op=OP.mult)
    var = sb.tile([H, 1], F32, tag=f"{tag}_var")
    nc.vector.tensor_tensor(out=var[:], in0=m[:, 1:2], in1=msq[:], op=OP.subtract)
    nc.vector.tensor_scalar_add(var[:], var[:], BN_EPS)
    sd = sb.tile([H, 1], F32, tag=f"{tag}_sd")
    nc.scalar.activation(out=sd[:], in_=var[:], func=AF.Sqrt)
    inv = sb.tile([H, 1], F32, tag=f"{tag}_inv")
    nc.vector.reciprocal(out=inv[:], in_=sd[:])
    scale = sb.tile([H, 1], F32, tag=f"{tag}_scale")
    nc.vector.tensor_tensor(out=scale[:], in0=inv[:], in1=bng[:], op=OP.mult)
    t = sb.tile([H, 1], F32, tag=f"{tag}_t")
    nc.vector.tensor_tensor(out=t[:], in0=m[:, 0:1], in1=scale[:], op=OP.mult)
    shift = sb.tile([H, 1], F32, tag=f"{tag}_shift")
    nc.vector.tensor_tensor(out=shift[:], in0=bnb[:], in1=t[:], op=OP.subtract)
    return scale, shift


def _node_phase(nc, tc, sb, psn, rt_sb, aggT, sh, n_total, cnt_th, W, j,
                feat_st, out_cb):
    """agg = R@g2W + cnt x g2b; BN stats -> AllReduce -> coeffs; then per-chunk
    BN-apply + f-MLP; out_cb(n0, m, psum_out, sb) consumes the f2 psum."""
    nchunk = (sh + 499) // 500
    sum_acc = sb.tile([H, nchunk], F32, tag=f"sum_acc{j}")
    sq_acc = sb.tile([H, nchunk], F32, tag=f"sq_acc{j}")
    for ci, n0 in enumerate(range(0, sh, 500)):
        m = min(500, sh - n0)
        cntc = sb.tile([1, 500], F32, tag=f"cntc{j}")
        nc.sync.dma_start(out=cntc[:, :m], in_=cnt_th.ap()[:, n0:n0 + m])
        pa = psn.tile([H, 500], F32, space="PSUM", tag="ps_agg")
        nc.tensor.matmul(out=pa[:, :m], lhsT=W[f"g2W{j}"][:],
                         rhs=rt_sb[:, n0:n0 + m], start=True, stop=False)
        nc.tensor.matmul(out=pa[:, :m], lhsT=W[f"g2b{j}"][:],
                         rhs=cntc[:, :m], start=False, stop=True)
        nc.vector.tensor_copy(out=aggT[:, n0:n0 + m], in_=pa[:, :m])
        nc.vector.tensor_reduce(out=sum_acc[:, ci:ci + 1],
                                in_=aggT[:, n0:n0 + m], axis=AX.X, op=OP.add)
        sq_scr = sb.tile([H, 500], F32, tag=f"sq_scr{j}")
        nc.scalar.activation(out=sq_scr[:, :m], in_=aggT[:, n0:n0 + m],
                             func=AF.Square)
        nc.vector.tensor_reduce(out=sq_acc[:, ci:ci + 1], in_=sq_scr[:, :m],
                                axis=AX.X, op=OP.add)
    stats = sb.tile([H, 2], F32, tag=f"stats{j}")
    nc.vector.tensor_reduce(out=stats[:, 0:1], in_=sum_acc[:], axis=AX.X,
                            op=OP.add)
    nc.vector.tensor_reduce(out=stats[:, 1:2], in_=sq_acc[:], axis=AX.X,
                            op=OP.add)
    return stats, nchunk


def build_program(com1, com2, phases="ABCDEF"):
    nc = bacc.Bacc("TRN2", target_bir_lowering=False, debug=False,
                   enable_asserts=False, num_devices=NCORES)

    TH = {}

    def inp(name, shape, dt=F32):
        TH[name] = nc.dram_tensor(name, list(shape), dt, kind="ExternalInput")
        return TH[name]

    for nm, shape in [("vW", (9, H)), ("vb", (H, 1)), ("cW", (5, H)),
                      ("cb", (H, 1)), ("hW1", (66, H)), ("hb1", (H, 1)),
                      ("hW2", (H, 1)), ("hb2", (1, 1))]:
        inp(nm, shape)
    for j in (1, 2):
        for nm, shape in [(f"g1Wa{j}", (H, H)), (f"g1Wb{j}", (H, H)),
                          (f"g1b{j}", (1, H)), (f"g2W{j}", (H, H)),
                          (f"g2b{j}", (1, H)), (f"bng{j}", (H, 1)),
                          (f"bnb{j}", (H, 1)), (f"f1Wt{j}", (H, H)),
                          (f"f1Wb{j}", (H, H)), (f"f1b{j}", (H, 1)),
                          (f"f2W{j}", (H, H)), (f"f2b{j}", (1, H))]:
            inp(nm, shape)
        inp(f"ug{j}", (P, H), BF)
    inp("iota", (P, P), BF)
    inp("xvT", (9, NV))
    inp("xvT_sh", (9, SHV))
    inp("xcT_sh", (5, SHC))
    inp("cand", (2, 1))
    for j, com in ((1, com1), (2, com2)):
        NG = com["NG"]
        inp(f"srcw{j}", (NG, P, GT * 8), I16)
        inp(f"dstw{j}", (NG, P, GT * 8), I16)
        inp(f"ea{j}", (NG, P, GT))
        inp(f"rel{j}", (NG, P, GT), BF)
        inp(f"cnt{j}", (1, com["sh"]))

    score_out = nc.dram_tensor("score", [1, 1], F32, kind="ExternalOutput")
    grp = [list(range(NCORES))]

    with tile.TileContext(nc) as tc:
        with tc.tile_pool(name="dram", bufs=1, space="DRAM") as dram, \
             tc.tile_pool(name="const", bufs=1) as const:
            # DRAM scratch
            # dma_gather requires offset-0 source tensors: one per 25000-row
            # chunk instead of views into a big table.
            P_v_c = [dram.tile([CHUNK, 128], BF, name=f"pvc{c}", tag=f"pvc{c}")
                     for c in range(4)]
            Pd_c = dram.tile([SHC, 128], BF)
            Pd_v = dram.tile([SHV, 128], BF)
            vT_st = dram.tile([H, SHV], F32)
            cT_st = dram.tile([H, SHC], F32)
            P2_loc = dram.tile([SHC, 128], BF)
            P2_full = dram.tile([NCN, 128], BF)
            P2_hi = dram.tile([CHUNK, 128], BF)
            cc_in1 = dram.tile([H, 2], F32)
            cc_out1 = dram.tile([H, 2], F32)
            cc_in2 = dram.tile([H, 2], F32)
            cc_out2 = dram.tile([H, 2], F32)
            cc_inp = dram.tile([H, 1], F32)
            cc_outp = dram.tile([H, 1], F32)

            W = {}
            for nm, th in TH.items():
                if nm in ("xvT", "xvT_sh", "xcT_sh", "cand") or nm[:3] in (
                        "src", "dst") or nm[:2] in ("ea", "re", "cn"):
                    continue
                t_sb = const.tile(list(th.shape), th.dtype, tag=f"w_{nm}")
                nc.sync.dma_start(out=t_sb[:], in_=th.ap())
                W[nm] = t_sb
            ones_row = const.tile([1, 512], F32, tag="ones")
            nc.vector.memset(ones_row[:], 1.0)

            # ---------------- Phase A ----------------
            def proj_pass(sbA, psA, ft, m, projW, projb, ptab, row0):
                # ptab: single tile, or list of CHUNK-row tiles (row0 global)
                for j0 in range(0, m, WIN):
                    mj = min(WIN, m - j0)
                    ps2 = psA.tile([P, H], F32, space="PSUM", tag="ps_proj")
                    nc.tensor.matmul(out=ps2[:mj, :], lhsT=ft[:, j0:j0 + mj],
                                     rhs=projW[:], start=True,
                                     stop=projb is None)
                    if projb is not None:
                        nc.tensor.matmul(out=ps2[:mj, :],
                                         lhsT=ones_row[:, :mj], rhs=projb[:],
                                         start=False, stop=True)
                    st = sbA.tile([P, H], BF, tag="st_proj")
                    nc.vector.tensor_copy(out=st[:mj, :], in_=ps2[:mj, :])
                    r = row0 + j0
                    if isinstance(ptab, list):
                        tgt = ptab[r // CHUNK]
                        r = r % CHUNK
                    else:
                        tgt = ptab
                    nc.sync.dma_start(out=tgt[r:r + mj, :H], in_=st[:mj, :])

            with tc.tile_pool(name="sbA", bufs=3) as sbA, \
                 tc.tile_pool(name="slabA", bufs=2) as slabA, \
                 tc.tile_pool(name="psA", bufs=2, space="PSUM") as psA:
                if "A" not in phases:
                    raise ValueError("phase A required")
                SLAB = 10000
                for s0 in range(0, NV, SLAB):
                    sm = min(SLAB, NV - s0)
                    slab = slabA.tile([9, SLAB], F32, tag="xv_slab")
                    nc.sync.dma_start(out=slab[:, :sm],
                                      in_=TH["xvT"].ap()[:, s0:s0 + sm])
                    for n0 in range(0, sm, 500):
                        m = min(500, sm - n0)
                        ps1 = psA.tile([H, 500], F32, space="PSUM", tag="ps_i")
                        nc.tensor.matmul(out=ps1[:, :m], lhsT=W["vW"][:],
                                         rhs=slab[:, n0:n0 + m], start=True,
                                         stop=True)
                        ft = sbA.tile([H, 500], F32, tag="ft")
                        nc.scalar.activation(out=ft[:, :m], in_=ps1[:, :m],
                                             func=AF.Relu, bias=W["vb"][:])
                        proj_pass(sbA, psA, ft, m, W["g1Wa1"], None, P_v_c,
                                  s0 + n0)
                # var shard -> vT_st + Pd_v
                for n0 in range(0, SHV, 500):
                    m = min(500, SHV - n0)
                    xsl = sbA.tile([9, 500], F32, tag="xsl")
                    nc.sync.dma_start(out=xsl[:, :m],
                                      in_=TH["xvT_sh"].ap()[:, n0:n0 + m])
                    ps1 = psA.tile([H, 500], F32, space="PSUM", tag="ps_i")
                    nc.tensor.matmul(out=ps1[:, :m], lhsT=W["vW"][:],
                                     rhs=xsl[:, :m], start=True, stop=True)
                    ft = sbA.tile([H, 500], F32, tag="ft")
                    nc.scalar.activation(out=ft[:, :m], in_=ps1[:, :m],
                                         func=AF.Relu, bias=W["vb"][:])
                    nc.sync.dma_start(out=vT_st[:, n0:n0 + m], in_=ft[:, :m])
                    proj_pass(sbA, psA, ft, m, W["g1Wb2"], W["g1b2"], Pd_v, n0)
                # con shard -> cT_st + Pd_c
                for n0 in range(0, SHC, 500):
                    m = min(500, SHC - n0)
                    xsl = sbA.tile([5, 500], F32, tag="xslc")
                    nc.sync.dma_start(out=xsl[:, :m],
                                      in_=TH["xcT_sh"].ap()[:, n0:n0 + m])
                    ps1 = psA.tile([H, 500], F32, space="PSUM", tag="ps_i")
                    nc.tensor.matmul(out=ps1[:, :m], lhsT=W["cW"][:],
                                     rhs=xsl[:, :m], start=True, stop=True)
                    ft = sbA.tile([H, 500], F32, tag="ft")
                    nc.scalar.activation(out=ft[:, :m], in_=ps1[:, :m],
                                         func=AF.Relu, bias=W["cb"][:])
                    nc.sync.dma_start(out=cT_st[:, n0:n0 + m], in_=ft[:, :m])
                    proj_pass(sbA, psA, ft, m, W["g1Wb1"], W["g1b1"], Pd_c, n0)

            # ---------------- conv1 ----------------
            with tc.tile_pool(name="rt1p", bufs=1) as rt1p:
                if "B" in phases:
                    rt1 = rt1p.tile([H, com1["nwin"] * 128], F32, tag="rt1")
                    with tc.tile_pool(name="workB", bufs=3) as workB, \
                         tc.tile_pool(name="psB", bufs=4, space="PSUM") as psB:
                        ins1 = {k: TH[f"{k}1"]
                                for k in ("srcw", "dstw", "ea", "rel")}
                        views1 = [P_v_c[c][:] for c in range(4)]
                        _edge_phase(nc, tc, workB, psB, com1, ins1, views1,
                                    Pd_c[:], W["ug1"], W["iota"], rt1)

                if "C" in phases:
                  with tc.tile_pool(name="c1p", bufs=1) as c1p, \
                     tc.tile_pool(name="sbC", bufs=3) as sbC, \
                     tc.tile_pool(name="psC", bufs=2, space="PSUM") as psC:
                    lvl = 9
                    for ch in phases:
                        if ch.isdigit():
                            lvl = int(ch)
                    aggT1 = c1p.tile([H, SHC], F32, tag="aggT1")
                    stats, _ = _node_phase(nc, tc, sbC, psC, rt1, aggT1, SHC,
                                           NCN, TH["cnt1"], W, 1, cT_st, None)
                    if lvl >= 2:
                        nc.gpsimd.dma_start(out=cc_in1[:], in_=stats[:])
                        nc.gpsimd.collective_compute(
                            "AllReduce", OP.add, replica_groups=grp,
                            ins=[cc_in1.opt()], outs=[cc_out1.opt()])
                        stats_g = sbC.tile([H, 2], F32, tag="stats1g")
                        nc.sync.dma_start(out=stats_g[:], in_=cc_out1[:])
                    sc1, sh1 = _bn_coeffs(nc, sbC, stats_g if lvl >= 2 else stats,
                                          W["bng1"], W["bnb1"], NCN, "bn1")
                    c2T = c1p.tile([H, SHC], F32, tag="c2T")
                    for n0 in range(0, SHC, 500) if lvl >= 3 else []:
                        m = min(500, SHC - n0)
                        aggbn = sbC.tile([H, 500], F32, tag="aggbn1")
                        nc.vector.tensor_scalar(
                            out=aggbn[:, :m], in0=aggT1[:, n0:n0 + m],
                            scalar1=sc1[:], scalar2=sh1[:], op0=OP.mult,
                            op1=OP.add)
                        ctc = sbC.tile([H, 500], F32, tag="ctc1")
                        nc.sync.dma_start(out=ctc[:, :m],
                                          in_=cT_st[:, n0:n0 + m])
                        ph = psC.tile([H, 500], F32, space="PSUM", tag="ps_h")
                        nc.tensor.matmul(out=ph[:, :m], lhsT=W["f1Wt1"][:],
                                         rhs=ctc[:, :m], start=True, stop=False)
                        nc.tensor.matmul(out=ph[:, :m], lhsT=W["f1Wb1"][:],
                                         rhs=aggbn[:, :m], start=False,
                                         stop=True)
                        hT = sbC.tile([H, 500], F32, tag="hT1")
                        nc.scalar.activation(out=hT[:, :m], in_=ph[:, :m],
                                             func=AF.Relu, bias=W["f1b1"][:])
                        pc2 = psC.tile([H, 500], F32, space="PSUM", tag="ps_c2")
                        nc.tensor.matmul(out=pc2[:, :m], lhsT=W["f2W1"][:],
                                         rhs=hT[:, :m], start=True, stop=False)
                        nc.tensor.matmul(out=pc2[:, :m], lhsT=W["f2b1"][:],
                                         rhs=ones_row[:, :m], start=False,
                                         stop=True)
                        nc.vector.tensor_copy(out=c2T[:, n0:n0 + m],
                                              in_=pc2[:, :m])
                    for j0 in range(0, SHC, WIN) if lvl >= 4 else []:
                        mj = min(WIN, SHC - j0)
                        pp = psC.tile([P, H], F32, space="PSUM", tag="ps_pj")
                        nc.tensor.matmul(out=pp[:mj, :],
                                         lhsT=c2T[:, j0:j0 + mj],
                                         rhs=W["g1Wa2"][:], start=True,
                                         stop=True)
                        st = sbC.tile([P, H], BF, tag="st_p2")
                        nc.vector.tensor_copy(out=st[:mj, :], in_=pp[:mj, :])
                        nc.sync.dma_start(out=P2_loc[j0:j0 + mj, :H],
                                          in_=st[:mj, :])
                    if lvl >= 5:
                        nc.gpsimd.collective_compute(
                            "AllGather", OP.bypass, replica_groups=grp,
                            ins=[P2_loc.opt()], outs=[P2_full.opt()])
                    # copy upper half into its own offset-0 tensor for gather
                    for r0 in range(0, CHUNK, 2048) if lvl >= 6 else []:
                        m = min(2048, CHUNK - r0)
                        mf = m - m % 128
                        bounce = sbC.tile([P, 16, 128], BF, tag="p2b")
                        if mf:
                            nb = mf // 128
                            src = P2_full[CHUNK + r0:CHUNK + r0 + mf, :]
                            nc.sync.dma_start(
                                out=bounce[:, :nb, :],
                                in_=src.rearrange("(b p) c -> p b c", p=128))
                            dst = P2_hi[r0:r0 + mf, :]
                            nc.sync.dma_start(
                                out=dst.rearrange("(b p) c -> p b c", p=128),
                                in_=bounce[:, :nb, :])
                        if m - mf:
                            rem = m - mf
                            bounce2 = sbC.tile([P, 128], BF, tag="p2b2")
                            nc.sync.dma_start(
                                out=bounce2[:rem, :],
                                in_=P2_full[CHUNK + r0 + mf:CHUNK + r0 + m, :])
                            nc.sync.dma_start(
                                out=P2_hi[r0 + mf:r0 + m, :],
                                in_=bounce2[:rem, :])

            # ---------------- conv2 ----------------
            with tc.tile_pool(name="rt2p", bufs=1) as rt2p:
                if "D" in phases:
                  rt2 = rt2p.tile([H, com2["nwin"] * 128], F32, tag="rt2")
                  with tc.tile_pool(name="workD", bufs=3) as workD, \
                     tc.tile_pool(name="psD", bufs=4, space="PSUM") as psD:
                    ins2 = {k: TH[f"{k}2"]
                            for k in ("srcw", "dstw", "ea", "rel")}
                    views2 = [P2_full[0:CHUNK, :], P2_hi[:]]
                    _edge_phase(nc, tc, workD, psD, com2, ins2, views2,
                                Pd_v[:], W["ug2"], W["iota"], rt2)

                if "E" in phases:
                  with tc.tile_pool(name="e2p", bufs=1) as e2p, \
                     tc.tile_pool(name="sbE", bufs=3) as sbE, \
                     tc.tile_pool(name="psE", bufs=2, space="PSUM") as psE:
                    aggT2 = e2p.tile([H, SHV], F32, tag="aggT2")
                    stats2, nchunk2 = _node_phase(nc, tc, sbE, psE, rt2, aggT2,
                                                  SHV, NV, TH["cnt2"], W, 2,
                                                  vT_st, None)
                    nc.gpsimd.dma_start(out=cc_in2[:], in_=stats2[:])
                    nc.gpsimd.collective_compute(
                        "AllReduce", OP.add, replica_groups=grp,
                        ins=[cc_in2.opt()], outs=[cc_out2.opt()])
                    stats2_g = sbE.tile([H, 2], F32, tag="stats2g")
                    nc.sync.dma_start(out=stats2_g[:], in_=cc_out2[:])
                    sc2, sh2 = _bn_coeffs(nc, sbE, stats2_g, W["bng2"],
                                          W["bnb2"], NV, "bn2")
                    pool_acc = sbE.tile([H, nchunk2], F32, tag="pool_acc")
                    for ci, n0 in enumerate(range(0, SHV, 500)):
                        m = min(500, SHV - n0)
                        aggbn = sbE.tile([H, 500], F32, tag="aggbn2")
                        nc.vector.tensor_scalar(
                            out=aggbn[:, :m], in0=aggT2[:, n0:n0 + m],
                            scalar1=sc2[:], scalar2=sh2[:], op0=OP.mult,
                            op1=OP.add)
                        vtc = sbE.tile([H, 500], F32, tag="vtc")
                        nc.sync.dma_start(out=vtc[:, :m],
                                          in_=vT_st[:, n0:n0 + m])
                        ph = psE.tile([H, 500], F32, space="PSUM", tag="ps_h")
                        nc.tensor.matmul(out=ph[:, :m], lhsT=W["f1Wt2"][:],
                                         rhs=vtc[:, :m], start=True, stop=False)
                        nc.tensor.matmul(out=ph[:, :m], lhsT=W["f1Wb2"][:],
                                         rhs=aggbn[:, :m], start=False,
                                         stop=True)
                        hT = sbE.tile([H, 500], F32, tag="hT2")
                        nc.scalar.activation(out=hT[:, :m], in_=ph[:, :m],
                                             func=AF.Relu, bias=W["f1b2"][:])
                        pv2 = psE.tile([H, 500], F32, space="PSUM", tag="ps_c2")
                        nc.tensor.matmul(out=pv2[:, :m], lhsT=W["f2W2"][:],
                                         rhs=hT[:, :m], start=True, stop=False)
                        nc.tensor.matmul(out=pv2[:, :m], lhsT=W["f2b2"][:],
                                         rhs=ones_row[:, :m], start=False,
                                         stop=True)
                        nc.vector.tensor_reduce(out=pool_acc[:, ci:ci + 1],
                                                in_=pv2[:, :m], axis=AX.X,
                                                op=OP.add)
                    pool_l = sbE.tile([H, 1], F32, tag="pool_l")
                    nc.vector.tensor_reduce(out=pool_l[:], in_=pool_acc[:],
                                            axis=AX.X, op=OP.add)
                    nc.gpsimd.dma_start(out=cc_inp[:], in_=pool_l[:])
                    nc.gpsimd.collective_compute(
                        "AllReduce", OP.add, replica_groups=grp,
                        ins=[cc_inp.opt()], outs=[cc_outp.opt()])

            # ---------------- head ----------------
            if "F" not in phases:
                with tc.tile_pool(name="sbF0", bufs=1) as sbF0:
                    sc0 = sbF0.tile([1, 1], F32, tag="sc0")
                    nc.vector.memset(sc0[:], 0.0)
                    nc.sync.dma_start(out=score_out.ap(), in_=sc0[:])
            else:
              with tc.tile_pool(name="sbF", bufs=1) as sbF, \
                 tc.tile_pool(name="psF", bufs=1, space="PSUM") as psF:
                xh = sbF.tile([66, 1], F32, tag="xh")
                scr = sbF.tile([66, 1], F32, tag="scr")
                poolg = sbF.tile([H, 1], F32, tag="poolg")
                nc.sync.dma_start(out=poolg[:], in_=cc_outp[:])
                nc.vector.tensor_scalar_mul(xh[0:H, :], poolg[:], 1.0 / NV)
                nc.sync.dma_start(out=xh[H:66, :], in_=TH["cand"].ap())
                nc.scalar.activation(out=scr[H:66, :], in_=xh[H:66, :],
                                     func=AF.Abs)
                nc.vector.tensor_scalar_add(scr[H:66, :], scr[H:66, :], 1.0)
                nc.vector.reciprocal(out=scr[H:66, :], in_=scr[H:66, :])
                nc.vector.tensor_tensor(out=xh[H:66, :], in0=xh[H:66, :],
                                        in1=scr[H:66, :], op=OP.mult)
                ph1 = psF.tile([H, 1], F32, space="PSUM", tag="ps_h1")
                nc.tensor.matmul(out=ph1[:], lhsT=W["hW1"][:], rhs=xh[:],
                                 start=True, stop=True)
                h1 = sbF.tile([H, 1], F32, tag="h1")
                nc.scalar.activation(out=h1[:], in_=ph1[:], func=AF.Relu,
                                     bias=W["hb1"][:])
                ph2 = psF.tile([1, 1], F32, space="PSUM", tag="ps_h2")
                nc.tensor.matmul(out=ph2[:], lhsT=W["hW2"][:], rhs=h1[:],
                                 start=True, stop=True)
                sc = sbF.tile([1, 1], F32, tag="sc")
                nc.vector.tensor_tensor(out=sc[:], in0=ph2[:], in1=W["hb2"][:],
                                        op=OP.add)
                nc.sync.dma_start(out=score_out.ap(), in_=sc[:])

    nc.compile()
    return nc


# ----------------------------------------------------------------------------
# Entry point
# ----------------------------------------------------------------------------

def _prepare(x_var, x_con, edge_index, edge_attr, cand_scalars, params):
    ei = np.asarray(edge_index)
    con_idx = ei[0].astype(np.int64)
    var_idx = ei[1].astype(np.int64)
    ea = np.asarray(edge_attr, dtype=np.float32).reshape(-1)

    com1, pc1 = _prep_conv(con_idx, var_idx, ea, SHC, 4)
    com2, pc2 = _prep_conv(var_idx, con_idx, ea, SHV, 2)
    w = _prep_weights(params)

    xvT = np.ascontiguousarray(np.asarray(x_var, dtype=np.float32).T)
    xcT = np.ascontiguousarray(np.asarray(x_con, dtype=np.float32).T)
    cand = np.asarray(cand_scalars, dtype=np.float32).reshape(2, 1)

    in_maps = []
    for k in range(NCORES):
        m = dict(w)
        m["xvT"] = xvT
        m["xvT_sh"] = np.ascontiguousarray(xvT[:, k * SHV:(k + 1) * SHV])
        m["xcT_sh"] = np.ascontiguousarray(xcT[:, k * SHC:(k + 1) * SHC])
        m["cand"] = cand
        for j, pc in ((1, pc1), (2, pc2)):
            for nm in ("srcw", "dstw", "ea", "rel", "cnt"):
                m[f"{nm}{j}"] = pc[k][nm]
        in_maps.append(m)
    return com1, com2, in_maps


def kernel(x_var, x_con, edge_index, edge_attr, cand_scalars, params):
    com1, com2, in_maps = _prepare(x_var, x_con, edge_index, edge_attr,
                                   cand_scalars, params)
    nc = build_program(com1, com2)
    res = run_bass_kernel_spmd(nc, in_maps, core_ids=list(range(NCORES)))
    score = res.results[0]["score"][0, 0]
    return np.float32(score).reshape(())
